# revision 68
# baseline (speedup 1.0000x reference)
"""Trainium2 Bass kernel for nn_MemoryRetriever (cross-attention memory retriever).

Strategy:
  * Host-side mask compaction: only unmasked memory tokens (~50%) are shipped
    to the device.  Compacted keys are padded to a fixed capacity and sharded
    across 8 NeuronCores (SKC keys per core); pad keys get a -1e30 exp bias so
    they contribute exactly zero.
  * All matmuls bf16 (fp32 PSUM accumulation); softmax arithmetic fp32.
    (fp8/DoubleRow was measured and rejected: each fp8 tensor on the value
    path adds ~3e-2 output error against the 2e-2 gate.)
  * Q is sharded: each core projects/normalizes/rotates only its 64-query
    block, then an AllGather shares the blocks.  Rank-c's block is exactly
    queries c*64..c*64+63, so the gathered core-major layout preserves the
    original query order and nothing downstream changes.
  * Per-key RMSNorm scale is folded into the exp activation's per-partition
    fp32 scale AP; the per-key sum-of-squares is accumulated directly in
    partition layout via free N=1 matmuls (ysq_tile.T @ ones); the rsqrt runs
    as an affine seed + 2 Newton steps on tiny DVE ops, so the Act engine
    never switches activation tables mid-loop (exp only).
  * RoPE pair-swap via DVE stream_shuffle with the sign folded into the sin
    tables; RoPE result overwrites yk in place.
  * Software pipelining: chunk ci+1's K/V projection work is interleaved with
    chunk ci's scores/exp/attnV phase so the tensor engine fills the
    activation-limited stretches.  DMA emission order puts wk/chunk-0 first
    so the PE starts ~4us in; PSUM start=True clears a whole bank, so den +
    per-chunk ssq share one bank with a single whole-program clear and
    disjoint column ranges.
  * Each core computes local masked-softmax partials (numerator +
    denominator), one AllReduce combines them, then each core output-projects
    its own 64-query slice.
"""

import sys
import numpy as np

sys.path.insert(0, "/opt/trn_rl_repo")

DIM = 1024
HEADS = 8
HD = 128
SQ = 512
SK = 31290
N_CORES = 8
QS = SQ // N_CORES
EPS = 1e-6
SCALE = 1.0 / np.sqrt(128.0)
NEG = -1.0e30
BSHIFT = -2.0        # exp bias shift (cancels exactly in num/den ratio)
CHUNK_TILES = 4      # key tiles per chunk (cw = 512)

_cache = {}


def _build(TT):
    """Build + compile the SPMD program for TT key-tiles per core."""
    key = ("nc", TT)
    if key in _cache:
        return _cache[key]

    import concourse.bass as bass
    import concourse.tile as tile
    from concourse import mybir, bacc

    f32 = mybir.dt.float32
    bf16 = mybir.dt.bfloat16
    AF = mybir.ActivationFunctionType
    SKC = TT * 128
    SWAP_MASK = [i ^ 1 for i in range(32)]

    nc = bacc.Bacc("TRN2", target_bir_lowering=False, debug=False,
                   num_devices=N_CORES)

    def din(name, shape, dt=f32):
        return nc.dram_tensor(name, list(shape), dt, kind="ExternalInput").ap()

    # per-core sharded inputs
    memT = din("memT", [DIM, SKC], bf16)    # mem shard, feature-major
    cskt = din("cskt", [HD, 2, SKC], bf16)  # K rope cos|sin (sign folded)
    mbias = din("mbias", [128, TT])         # exp bias (BSHIFT real / -1e30 pad)
    # shared inputs
    xT = din("xT", [DIM, SQ], bf16)
    wq = din("wq", [128, 8, 8, 128], bf16)  # [p,i,o,m] = Wq.T[i*128+p, o*128+m]
    wk = din("wk", [128, 8, 8, 128], bf16)  # [p,o,i,m] (o-major for per-o DMA)
    wo = din("wo", [128, 8, 8, 128], bf16)  # [p,o,e,m] = Wo.T[o*128+p, e*128+m]
    wv = din("wv", [128, 8, DIM], bf16)     # [p,i,o] = Wv.T[i*128+p, o]
    ctq = din("ctq", [128, 8, SQ], bf16)    # q rope cos (gq folded)
    stq = din("stq", [128, 8, SQ], bf16)    # q rope sin (gq + sign folded)
    cpack = din("cpack", [128, 48])         # bq|bk|bo|bvh|rsab|eps (packed)

    outT = nc.dram_tensor("outT", [DIM, SQ], f32, kind="ExternalOutput").ap()

    import os as _os
    _sim = _os.environ.get("KSIM", "0") == "1"
    _dbg = _os.environ.get("KDBG", "0") == "1"
    if _dbg:
        qdbg = nc.dram_tensor("qdbg", [128, 8, SQ], bf16, kind="ExternalOutput").ap()
        ykdbg = nc.dram_tensor("ykdbg", [128, 8, 512], bf16, kind="ExternalOutput").ap()
        rsbdbg = nc.dram_tensor("rsbdbg", [128, CHUNK_TILES], f32, kind="ExternalOutput").ap()
        ptdbg = nc.dram_tensor("ptdbg", [128, SQ], bf16, kind="ExternalOutput").ap()
        ddbg = nc.dram_tensor("ddbg", [128, 8, 4], f32, kind="ExternalOutput").ap()
    qcat = nc.dram_tensor("qcat", [DIM, QS], bf16)
    qcat_sh = nc.dram_tensor("qcat_sh", [N_CORES, DIM, QS], bf16,
                             addr_space="Shared")
    cat = nc.dram_tensor("cat", [DIM + HEADS, SQ], f32)
    cat_sh = nc.dram_tensor("cat_sh", [DIM + HEADS, SQ], f32,
                            addr_space="Shared")

    with tile.TileContext(nc) as tc:
        ctx_pools = []

        def pool(name, bufs, space=None):
            kw = dict(name=name, bufs=bufs)
            if space:
                kw["space"] = space
            p = tc.tile_pool(**kw)
            ctx_pools.append(p)
            return p.__enter__()

        consts = pool("consts", 1)
        resid = pool("resid", 1)
        pp = pool("pp", 3, space="PSUM")          # projection transients
        ppa = pool("ppa", 2, space="PSUM")        # scores
        ppn = pool("ppn", 2, space="PSUM")        # attnV accumulation / Q ssq
        ppd = pool("ppd", 1, space="PSUM")        # den + per-chunk ssq

        # ---- all small constants in one DMA ----
        cp = consts.tile([128, 48], f32)
        bq_s = cp[:, 0:8]
        bk_s = cp[:, 8:16]
        bo_s2 = cp[:, 16:24]
        bvh_s2 = cp[:, 24:32]
        rsab_s = cp[:, 32:34]
        epsq_s = cp[0:1, 34:35]
        ones_s = consts.tile([128, 1], bf16)
        nc.vector.memset(ones_s[:], 1.0)
        qT = resid.tile([128, N_CORES, 8, QS], bf16)  # gathered Q [p,core,o,j]
        den_sb = resid.tile([128, HEADS, 4], f32)
        pid = nc.sync.partition_id()
        qoff = pid * QS

        # single bank, single whole-bank clear (chunk 0's first ssq MM):
        # den in cols 0:32 (never uses start=True; accumulates into the
        # pending-zero region), per-chunk ssq in disjoint cols 32+4ci..
        denbank = ppd.tile([128, 128], f32, tag="den")
        den_ps = denbank[:, 0:32]

        # ---- DMA head ordering: wk (per-o) + chunk0 first so the tensor
        # engine starts on K-projection ~7us in; xT/Wq stream during it ----
        kpool = pool("kpool", 3)
        qpool_cm = tc.tile_pool(name="qpool", bufs=1)
        qpool = qpool_cm.__enter__()
        wk_s = resid.tile([128, 8, 8, 128], bf16)   # [p, o, i, m]
        for o in range(2):
            nc.sync.dma_start(wk_s[:, o, :, :], wk[:, o, :, :])

        # chunk layout
        csizes = [2] + [4] * ((TT - 8) // 4) + [2, 2, 1, 1]
        assert sum(csizes) == TT
        nch = len(csizes)
        cstarts = [sum(csizes[:i]) for i in range(nch)]

        chunk_tiles = {}

        def emit_chunk_dma(ci):
            ntt = csizes[ci]
            cw = ntt * 128
            c0 = cstarts[ci] * 128
            memt = kpool.tile([128, 8, CHUNK_TILES * 128], bf16, tag="memt")
            nc.sync.dma_start(
                memt[:, :, 0:cw],
                memT[:, c0:c0 + cw].rearrange("(i p) t -> p i t", p=128))
            cs_t = kpool.tile([128, 2, CHUNK_TILES * 128], bf16, tag="cskt")
            nc.sync.dma_start(cs_t[:, :, 0:cw], cskt[:, :, c0:c0 + cw])
            chunk_tiles[ci] = dict(memt=memt, ctk=cs_t[:, 0, :], stk=cs_t[:, 1, :])

        emit_chunk_dma(0)
        nc.sync.dma_start(cp[:], cpack)
        for o in range(2, 8):
            nc.sync.dma_start(wk_s[:, o, :, :], wk[:, o, :, :])
        wv_s = resid.tile([128, 8, DIM], bf16)
        nc.sync.dma_start(wv_s[:], wv)
        xt_s = qpool.tile([128, 8, QS], bf16, tag="xt")
        nc.sync.dma_start(
            xt_s[:],
            xT.rearrange("(i p) q -> p i q", p=128)[:, :, bass.ds(qoff, QS)])
        wq_all = qpool.tile([128, 8, 8, 128], bf16, tag="wq_all")
        nc.sync.dma_start(wq_all[:], wq)
        qpf_cm = tc.tile_pool(name="qpf", bufs=1)
        qpf = qpf_cm.__enter__()
        ctq_all = qpf.tile([128, 8, QS], bf16, tag="ctq_all")
        nc.sync.dma_start(ctq_all[:], ctq[:, :, bass.ds(qoff, QS)])
        stq_all = qpf.tile([128, 8, QS], bf16, tag="stq_all")
        nc.sync.dma_start(stq_all[:], stq[:, :, bass.ds(qoff, QS)])
        mb_s = consts.tile([128, TT], f32)
        nc.sync.dma_start(mb_s[:], mbias)
        if nch > 1:
            emit_chunk_dma(1)
        if nch > 2:
            emit_chunk_dma(2)

        MULT = mybir.AluOpType.mult
        ADD = mybir.AluOpType.add

        def produce_pieces(ci):
            """K proj + rsqrt + rope + V proj for chunk ci, as a list of
            emission callables (interleaved with the previous chunk's
            consume phase)."""
            ntt = csizes[ci]
            cw = ntt * 128
            ct = chunk_tiles[ci]
            memt, ctk_t, stk_t = ct["memt"], ct["ctk"], ct["stk"]
            yk = kpool.tile([128, 8, CHUNK_TILES * 128], bf16, tag="yk")
            ct["kr"] = yk                       # rope overwrites in place
            v_sb = kpool.tile([128, CHUNK_TILES, DIM], bf16, tag="v")
            ct["v"] = v_sb
            rsb_t = kpool.tile([128, CHUNK_TILES], f32, tag="rsbt")
            ct["rsb"] = rsb_t
            pieces = []

            def kproj(o):
                ps_y = pp.tile([128, 512], f32, tag="ps")
                for i in range(8):
                    nc.tensor.matmul(ps_y[:, 0:cw], wk_s[:, o, i, :],
                                     memt[:, i, 0:cw],
                                     start=(i == 0), stop=(i == 7))
                nc.vector.tensor_scalar_add(yk[:, o, 0:cw], ps_y[:, 0:cw],
                                            bk_s[:, o:o + 1])
                ysq = kpool.tile([128, CHUNK_TILES * 128], bf16, tag="ysq")
                nc.gpsimd.tensor_mul(ysq[:, 0:cw], yk[:, o, 0:cw],
                                     yk[:, o, 0:cw])
                sb = 32 + 4 * ci
                for tt in range(ntt):
                    # start=True only on the program's very first ssq MM
                    # (clears the whole bank once); everything else in this
                    # bank accumulates into the pending-zero region.
                    nc.tensor.matmul(
                        denbank[:, sb + tt:sb + tt + 1],
                        ysq[:, tt * 128:(tt + 1) * 128], ones_s[:],
                        start=(ci == 0 and o == 0 and tt == 0), stop=(o == 7))

            def rsqrt():
                # v = ssq/8 + 128*eps;  y = a - b*v  (host-fitted affine seed)
                # then two Newton steps y <- y*(1.5 - 0.5*v*y^2) on DVE.
                vt = kpool.tile([128, CHUNK_TILES], f32, tag="vt")
                sb = 32 + 4 * ci
                nc.vector.tensor_scalar(vt[:, 0:ntt], denbank[:, sb:sb + ntt],
                                        0.125, 128.0 * EPS, MULT, ADD)
                nc.vector.tensor_scalar(rsb_t[:, 0:ntt], vt[:, 0:ntt],
                                        rsab_s[:, 1:2], rsab_s[:, 0:1],
                                        MULT, ADD)
                tn = kpool.tile([128, CHUNK_TILES], f32, tag="tn")
                for _ in range(2):
                    nc.vector.tensor_mul(tn[:, 0:ntt], rsb_t[:, 0:ntt],
                                         rsb_t[:, 0:ntt])
                    nc.vector.tensor_mul(tn[:, 0:ntt], tn[:, 0:ntt],
                                         vt[:, 0:ntt])
                    nc.vector.tensor_scalar(tn[:, 0:ntt], tn[:, 0:ntt],
                                            -0.5, 1.5, MULT, ADD)
                    nc.vector.tensor_mul(rsb_t[:, 0:ntt], rsb_t[:, 0:ntt],
                                         tn[:, 0:ntt])
                if _dbg and ci == 0:
                    nc.sync.dma_start(rsbdbg, rsb_t[:])

            def rope(o):
                shf = kpool.tile([128, CHUNK_TILES * 128], bf16, tag="shf")
                nc.vector.stream_shuffle(shf[:, 0:cw], yk[:, o, 0:cw],
                                         SWAP_MASK)
                t1 = kpool.tile([128, CHUNK_TILES * 128], bf16, tag="t1")
                nc.vector.tensor_mul(t1[:, 0:cw], yk[:, o, 0:cw],
                                     ctk_t[:, 0:cw])
                t2 = kpool.tile([128, CHUNK_TILES * 128], bf16, tag="t2")
                nc.vector.tensor_mul(t2[:, 0:cw], shf[:, 0:cw],
                                     stk_t[:, 0:cw])
                nc.vector.tensor_add(yk[:, o, 0:cw], t1[:, 0:cw], t2[:, 0:cw])

            def vproj(tt):
                for oh in range(2):
                    ps_v = pp.tile([128, 512], f32, tag="ps")
                    for i in range(8):
                        nc.tensor.matmul(
                            ps_v[:], memt[:, i, tt * 128:(tt + 1) * 128],
                            wv_s[:, i, oh * 512:(oh + 1) * 512],
                            start=(i == 0), stop=(i == 7))
                    nc.scalar.activation(v_sb[:, tt, oh * 512:(oh + 1) * 512],
                                         ps_v[:], AF.Identity)

            pieces.append(lambda: (kproj(0), kproj(1)))
            pieces.append(lambda: (kproj(2), kproj(3)))
            pieces.append(lambda: (kproj(4), kproj(5)))
            pieces.append(lambda: (kproj(6), kproj(7), rsqrt()))
            pieces.append(lambda: (rope(0), rope(1), rope(2), rope(3)))
            pieces.append(lambda: (rope(4), rope(5), rope(6), rope(7)))
            pieces.append(lambda: tuple(vproj(t) for t in range(0, min(2, ntt))))
            if ntt > 2:
                pieces.append(lambda: tuple(vproj(t) for t in range(2, ntt)))
            return pieces

        def consume_pieces(ci):
            """scores -> exp -> attnV + den for chunk ci."""
            ntt = csizes[ci]
            ct0 = cstarts[ci]
            ct = chunk_tiles[ci]
            kr, v_sb, rsb_t = ct["kr"], ct["v"], ct["rsb"]
            state = {}
            last = ci == nch - 1

            def emit_numden(h, pts):
                ps_n = ppn.tile([128, SQ], f32, tag="psn")
                for tt in range(ntt):
                    nc.tensor.matmul(
                        ps_n[:], v_sb[:, tt, h * 128:(h + 1) * 128],
                        pts[tt][:], start=(tt == 0), stop=(tt == ntt - 1))
                for tt in range(ntt):
                    for qs in range(4):
                        nc.tensor.matmul(
                            den_ps[:, h * 4 + qs:h * 4 + qs + 1],
                            pts[tt][:, qs * 128:(qs + 1) * 128], ones_s[:],
                            start=False,
                            stop=(ci == nch - 1 and h == 7 and tt == ntt - 1
                                  and qs == 3))
                if ci == 0:
                    nc.vector.tensor_copy(nacc[:, h, :], ps_n[:])
                else:
                    nc.vector.tensor_add(nacc[:, h, :], nacc[:, h, :], ps_n[:])
                if last:
                    # numerator for this head is final: ship its cat slice now
                    if _sim:
                        nc.sync.dma_start(cat_sh[h * 128:(h + 1) * 128, :],
                                          nacc[:, h, :])
                    else:
                        nc.sync.dma_start(cat[h * 128:(h + 1) * 128, :],
                                          nacc[:, h, :])

            def head(h):
                pts = []
                for tt in range(ntt):
                    gtt = ct0 + tt
                    ps_s = ppa.tile([128, SQ], f32, tag="psa")
                    nc.tensor.matmul(ps_s[:], kr[:, h, tt * 128:(tt + 1) * 128],
                                     qT[:, :, h, :])
                    pt = ppool.tile([128, SQ], bf16, tag="pt")
                    nc.scalar.activation(pt[:], ps_s[:], AF.Exp,
                                         bias=mb_s[:, gtt:gtt + 1],
                                         scale=rsb_t[:, tt:tt + 1])
                    if _dbg and ci == 0 and h == 0 and tt == 0:
                        nc.sync.dma_start(ptdbg, pt[:])
                    pts.append(pt)
                prev = state.pop("prev", None)
                if prev is not None:
                    emit_numden(*prev)
                state["prev"] = (h, pts)

            def drain():
                prev = state.pop("prev", None)
                if prev is not None:
                    emit_numden(*prev)

            def attn_piece():
                prev = state.pop("prev", None)
                if prev is not None:
                    emit_numden(*prev)

            def head_scores(h):
                pts = []
                for tt in range(ntt):
                    gtt = ct0 + tt
                    ps_s = ppa.tile([128, SQ], f32, tag="psa")
                    nc.tensor.matmul(ps_s[:], kr[:, h, tt * 128:(tt + 1) * 128],
                                     qT[:, :, h, :])
                    pt = ppool.tile([128, SQ], bf16, tag="pt")
                    nc.scalar.activation(pt[:], ps_s[:], AF.Exp,
                                         bias=mb_s[:, gtt:gtt + 1],
                                         scale=rsb_t[:, tt:tt + 1])
                    pts.append(pt)
                state["prev"] = (h, pts)

            pieces = []
            for h in range(8):
                pieces.append(lambda h=h: head_scores(h))
                pieces.append(attn_piece)
            pieces.append(drain)
            return pieces

        for piece in produce_pieces(0):
            piece()

        # =========== Q phase ===========
        yq = qpool.tile([128, 8, QS], bf16, tag="yq")
        qT_own = qpool.tile([128, 8, QS], bf16, tag="qown")
        psq = ppn.tile([128, SQ], f32, tag="psn")   # partition-0 row holds ssq
        for o in range(8):
            ps_q = pp.tile([128, SQ], f32, tag="ps")
            for i in range(8):
                nc.tensor.matmul(ps_q[:, 0:QS], wq_all[:, i, o, :],
                                 xt_s[:, i, :], start=(i == 0), stop=(i == 7))
            nc.scalar.activation(yq[:, o, :], ps_q[:, 0:QS], AF.Identity,
                                 bias=bq_s[:, o:o + 1])
            ysq = qpool.tile([128, QS], bf16, tag="ysq")
            nc.vector.tensor_mul(ysq[:], yq[:, o, :], yq[:, o, :])
            nc.tensor.matmul(psq[0:1, 0:QS], ones_s[:], ysq[:],
                             start=(o == 0), stop=(o == 7))
        sq_q = qpool.tile([1, QS], f32, tag="sqr")
        nc.scalar.activation(sq_q[:], psq[0:1, 0:QS], AF.Sqrt,
                             bias=epsq_s[:], scale=1.0 / DIM)
        rs_q = qpool.tile([1, QS], f32, tag="rs")
        nc.vector.reciprocal(rs_q[:], sq_q[:])
        rsb_q = qpool.tile([128, QS], f32, tag="rsb")
        nc.gpsimd.partition_broadcast(rsb_q[:], rs_q[:])
        for o in range(8):
            shf = qpool.tile([128, QS], bf16, tag="shf")
            nc.vector.stream_shuffle(shf[:], yq[:, o, :], SWAP_MASK)
            t1 = qpool.tile([128, QS], bf16, tag="t1")
            nc.vector.tensor_mul(t1[:], yq[:, o, :], ctq_all[:, o, :])
            t2 = qpool.tile([128, QS], bf16, tag="t2")
            nc.vector.tensor_mul(t2[:], shf[:], stq_all[:, o, :])
            nc.vector.tensor_add(t1[:], t1[:], t2[:])
            nc.vector.tensor_mul(qT_own[:, o, :], t1[:], rsb_q[:])
        # gather all cores' q blocks (block c == original queries c*QS..)
        if _sim:
            nc.sync.dma_start(qcat_sh[bass.ds(pid, 1), :, :].rearrange(
                "c (o p) j -> p c o j", p=128), qT_own[:])
        else:
            nc.sync.dma_start(qcat.rearrange("(o p) q -> p o q", p=128),
                              qT_own[:])
            nc.gpsimd.collective_compute(
                "AllGather", mybir.AluOpType.bypass,
                replica_groups=[list(range(N_CORES))],
                ins=[qcat[:]], outs=[qcat_sh[:]])
        nc.sync.dma_start(
            qT[:], qcat_sh.rearrange("c (o p) j -> p c o j", p=128))
        qpf_cm.__exit__(None, None, None)
        qpool_cm.__exit__(None, None, None)
        ppool = pool("ppool", 24)
        late = pool("late", 1)
        nacc = late.tile([128, 8, SQ], f32)     # numerator accumulator

        prevC = consume_pieces(0)
        for ci in range(1, nch):
            if ci + 2 < nch:
                emit_chunk_dma(ci + 2)
            if ci == 1:
                # out-projection weights: load mid-loop so the tail needn't wait
                wo_s = late.tile([128, 8, 8, 128], bf16)
                nc.sync.dma_start(wo_s[:], wo)
            P = produce_pieces(ci)
            C = prevC
            # proportional interleave (consume leads)
            acc = 0.0
            ratio = len(P) / max(len(C), 1)
            pi = 0
            for c in C:
                acc += ratio
                while pi < len(P) and acc >= pi + 1:
                    P[pi]()
                    pi += 1
                c()
            while pi < len(P):
                P[pi]()
                pi += 1
            prevC = consume_pieces(ci)
        for c in prevC:
            c()
        if nch == 1:
            wo_s = late.tile([128, 8, 8, 128], bf16)
            nc.sync.dma_start(wo_s[:], wo)

        # =========== reduce across cores ===========
        nc.vector.tensor_copy(den_sb[:], den_ps[:])
        if _dbg:
            nc.sync.dma_start(ddbg, den_sb[:])
        if _sim:
            nc.sync.dma_start(
                cat_sh[DIM:DIM + HEADS, :].rearrange("h (qs p) -> p h qs", p=128),
                den_sb[:])
        else:
            nc.sync.dma_start(
                cat[DIM:DIM + HEADS, :].rearrange("h (qs p) -> p h qs", p=128),
                den_sb[:])
            nc.gpsimd.collective_compute(
                "AllReduce", mybir.AluOpType.add,
                replica_groups=[list(range(N_CORES))],
                ins=[cat[:]], outs=[cat_sh[:]])

        # =========== per-core output projection on its query slice ===========
        tail = pool("tail", 1)
        nred = tail.tile([128, 8, QS], f32)
        dred = tail.tile([1, HEADS, QS], f32)
        nc.sync.dma_start(
            nred[:],
            cat_sh[0:DIM, bass.ds(qoff, QS)].rearrange("(h p) q -> p h q", p=128))
        nc.sync.dma_start(dred[:], cat_sh[DIM:DIM + HEADS, bass.ds(qoff, QS)])
        rd = tail.tile([1, HEADS, QS], f32)
        nc.vector.reciprocal(rd[:], dred[:])
        nsc = tail.tile([128, 8, QS], bf16)
        rdb = tail.tile([128, 8, QS], f32, tag="rdb")
        nc.gpsimd.partition_broadcast(rdb[:], rd[0:1, :, :])
        nc.vector.tensor_mul(nsc[:], nred[:], rdb[:])
        for h in range(8):
            nc.vector.tensor_scalar_add(nsc[:, h, :], nsc[:, h, :],
                                        bvh_s2[:, h:h + 1])
        out_sb = tail.tile([128, 8, QS], f32)
        for e in range(8):
            ps_o = pp.tile([128, QS], f32, tag="ps")
            for o in range(8):
                nc.tensor.matmul(ps_o[:], wo_s[:, o, e, :], nsc[:, o, :],
                                 start=(o == 0), stop=(o == 7))
            nc.scalar.activation(out_sb[:, e, :], ps_o[:], AF.Identity,
                                 bias=bo_s2[:, e:e + 1])
        nc.sync.dma_start(
            outT.rearrange("(e p) q -> p e q", p=128)[:, :, 0:QS], out_sb[:])

        for p in reversed(ctx_pools):
            p.__exit__(None, None, None)

    nc.compile()
    _cache[key] = nc
    return nc


def _prep(TT, x, mem, mask, cos_q, sin_q, cos_k, sin_k,
          Wq, bq, Wk, bk, Wv, bv, Wo, bo, gq, gk):
    import ml_dtypes
    f = np.float32
    bf = ml_dtypes.bfloat16
    SKC = TT * 128
    CAP = N_CORES * SKC
    x = np.asarray(x, f).reshape(SQ, DIM)
    mem = np.asarray(mem, f).reshape(SK, DIM)
    mask = np.asarray(mask).reshape(SK)
    cos_q = np.asarray(cos_q, f)
    sin_q = np.asarray(sin_q, f)
    cos_k = np.asarray(cos_k, f)
    sin_k = np.asarray(sin_k, f)
    Wq, Wk, Wv, Wo = (np.asarray(w, f) for w in (Wq, Wk, Wv, Wo))
    bq, bk, bv, bo, gq, gk = (np.asarray(v, f) for v in (bq, bk, bv, bo, gq, gk))

    if not np.allclose(gk, 1.0):
        gkp = gk.reshape(-1, 2)
        assert np.allclose(gkp[:, 0], gkp[:, 1]), "unsupported non-pairwise gk"

    # mask compaction: keep only unmasked keys
    sel = np.flatnonzero(mask)
    n = sel.size
    assert n <= CAP, f"unmasked keys {n} exceed capacity {CAP}"
    memc = mem[sel]
    cos_kc = cos_k[sel]
    sin_kc = sin_k[sel]

    def tile_w(WT):  # [1024,1024] (in,out of W.T) -> [p, i, o, m]
        return np.ascontiguousarray(
            WT.reshape(8, 128, 8, 128).transpose(1, 0, 2, 3)).astype(bf)

    ii = np.arange(128)
    jj = ii // 2
    partner = ii ^ 1
    # sign of the swapped term: even in-head dims get -sin, odd get +sin
    sgn = np.where(ii % 2 == 0, -1.0, 1.0).astype(f)

    # fold gq (and pairwise gk) into the q rope tables; sin pairs with
    # partner's gq; swap sign folded into the sin tables
    gq_t = (gq * gk).reshape(8, 128)
    gq_sin = (gq.reshape(8, 128)[:, partner] * gk.reshape(8, 128))
    cq = cos_q[:, jj].T                # [128, SQ]
    sq = sin_q[:, jj].T * sgn[:, None]
    ctq_h = np.ascontiguousarray(
        (cq[None, :, :] * gq_t[:, :, None]).transpose(1, 0, 2)).astype(bf)
    stq_h = np.ascontiguousarray(
        (sq[None, :, :] * gq_sin[:, :, None]).transpose(1, 0, 2)).astype(bf)

    # rsqrt Newton affine seed from the expected per-key ssq scale:
    # v = ssq/8 + 128*eps with E[ssq] ~ |Wk|_F^2 var(mem) + |bk|^2
    e_ssq = float((Wk ** 2).sum()) * float(mem.var()) + float((bk ** 2).sum())
    v0 = max(e_ssq * 0.125 + 128.0 * EPS, 1e-6)
    y0 = 1.0 / np.sqrt(v0)
    cpack_h = np.zeros((128, 48), f)
    cpack_h[:, 0:8] = bq.reshape(8, 128).T
    cpack_h[:, 8:16] = bk.reshape(8, 128).T
    cpack_h[:, 16:24] = bo.reshape(8, 128).T
    cpack_h[:, 24:32] = bv.reshape(8, 128).T
    cpack_h[:, 32] = 1.5 * y0        # a:  y1 = a + b*v with b negative
    cpack_h[:, 33] = -0.5 * y0 ** 3
    cpack_h[:, 34] = EPS

    shared = {
        "xT": np.ascontiguousarray(x.T).astype(bf),
        "wq": tile_w(Wq.T),
        "wk": np.ascontiguousarray(tile_w(Wk.T).transpose(0, 2, 1, 3)),
        "wo": tile_w(Wo.T),
        "wv": np.ascontiguousarray(
            Wv.T.reshape(8, 128, DIM).transpose(1, 0, 2)).astype(bf),
        "ctq": ctq_h, "stq": stq_h,
        "cpack": cpack_h,
    }

    memT_full = np.zeros((DIM, CAP), bf)
    memT_full[:, :n] = memc.T.astype(bf)
    cskt_full = np.zeros((HD, 2, CAP), bf)
    cskt_full[:, 0, :n] = cos_kc[:, jj].T
    cskt_full[:, 1, :n] = sin_kc[:, jj].T * sgn[:, None]
    mb_full = np.full(CAP, NEG, f)
    mb_full[:n] = BSHIFT

    in_maps = []
    for c in range(N_CORES):
        s = slice(c * SKC, (c + 1) * SKC)
        m = dict(shared)
        m["memT"] = np.ascontiguousarray(memT_full[:, s])
        m["cskt"] = np.ascontiguousarray(cskt_full[:, :, s])
        m["mbias"] = np.ascontiguousarray(mb_full[s].reshape(TT, 128).T)
        in_maps.append(m)
    return in_maps


def _tt_for(mask):
    n = int(np.count_nonzero(np.asarray(mask).reshape(-1)))
    for tt in (16, 24, 32):
        if n <= N_CORES * tt * 128:
            return tt
    raise AssertionError("mask popcount exceeds max capacity")


def kernel(**inputs):
    from concourse.bass_utils import run_bass_kernel_spmd
    TT = _tt_for(inputs["mask"])
    nc = _build(TT)
    in_maps = _prep(TT, **inputs)
    res = run_bass_kernel_spmd(nc, in_maps, list(range(N_CORES)))
    parts = [res.results[c]["outT"][:, 0:QS].T for c in range(N_CORES)]
    out = np.concatenate(parts, axis=0)
    return out[None].astype(np.float32)


# revision 71
# speedup vs baseline: 1.0034x; 1.0034x over previous
"""Trainium2 Bass kernel for nn_MemoryRetriever (cross-attention memory retriever).

Strategy:
  * Host-side mask compaction: only unmasked memory tokens (~50%) are shipped
    to the device.  Compacted keys are padded to a fixed capacity and sharded
    across 8 NeuronCores (SKC keys per core); pad keys get a -1e30 exp bias so
    they contribute exactly zero.
  * All matmuls bf16 (fp32 PSUM accumulation); softmax arithmetic fp32.
    (fp8/DoubleRow was measured and rejected: each fp8 tensor on the value
    path adds ~3e-2 output error against the 2e-2 gate.)
  * Q is sharded: each core projects/normalizes/rotates only its 64-query
    block, then an AllGather shares the blocks.  Rank-c's block is exactly
    queries c*64..c*64+63, so the gathered core-major layout preserves the
    original query order and nothing downstream changes.
  * Per-key RMSNorm scale is folded into the exp activation's per-partition
    fp32 scale AP; the per-key sum-of-squares is accumulated directly in
    partition layout via free N=1 matmuls (ysq_tile.T @ ones); the rsqrt runs
    as an affine seed + 2 Newton steps on tiny DVE ops, so the Act engine
    never switches activation tables mid-loop (exp only).
  * RoPE pair-swap via DVE stream_shuffle with the sign folded into the sin
    tables; RoPE result overwrites yk in place.
  * Software pipelining: chunk ci+1's K/V projection work is interleaved with
    chunk ci's scores/exp/attnV phase so the tensor engine fills the
    activation-limited stretches.  DMA emission order puts wk/chunk-0 first
    so the PE starts ~4us in; PSUM start=True clears a whole bank, so den +
    per-chunk ssq share one bank with a single whole-program clear and
    disjoint column ranges.
  * Each core computes local masked-softmax partials (numerator +
    denominator), one AllReduce combines them, then each core output-projects
    its own 64-query slice.
"""

import sys
import numpy as np

sys.path.insert(0, "/opt/trn_rl_repo")

DIM = 1024
HEADS = 8
HD = 128
SQ = 512
SK = 31290
N_CORES = 8
QS = SQ // N_CORES
EPS = 1e-6
SCALE = 1.0 / np.sqrt(128.0)
NEG = -1.0e30
BSHIFT = -2.0        # exp bias shift (cancels exactly in num/den ratio)
CHUNK_TILES = 4      # key tiles per chunk (cw = 512)

_cache = {}


def _build(TT):
    """Build + compile the SPMD program for TT key-tiles per core."""
    key = ("nc", TT)
    if key in _cache:
        return _cache[key]

    import concourse.bass as bass
    import concourse.tile as tile
    from concourse import mybir, bacc

    f32 = mybir.dt.float32
    bf16 = mybir.dt.bfloat16
    AF = mybir.ActivationFunctionType
    SKC = TT * 128
    SWAP_MASK = [i ^ 1 for i in range(32)]

    nc = bacc.Bacc("TRN2", target_bir_lowering=False, debug=False,
                   num_devices=N_CORES)

    def din(name, shape, dt=f32):
        return nc.dram_tensor(name, list(shape), dt, kind="ExternalInput").ap()

    # per-core sharded inputs
    memT = din("memT", [DIM, SKC], bf16)    # mem shard, feature-major
    cskt = din("cskt", [HD, 2, SKC], bf16)  # K rope cos|sin (sign folded)
    mbias = din("mbias", [128, TT])         # exp bias (BSHIFT real / -1e30 pad)
    # shared inputs
    xT = din("xT", [DIM, SQ], bf16)
    wq = din("wq", [128, 8, 8, 128], bf16)  # [p,i,o,m] = Wq.T[i*128+p, o*128+m]
    wk = din("wk", [128, 8, 8, 128], bf16)  # [p,o,i,m] (o-major for per-o DMA)
    wo = din("wo", [128, 8, 8, 128], bf16)  # [p,o,e,m] = Wo.T[o*128+p, e*128+m]
    wv = din("wv", [128, 8, DIM], bf16)     # [p,i,o] = Wv.T[i*128+p, o]
    ctq = din("ctq", [128, 8, SQ], bf16)    # q rope cos (gq folded)
    stq = din("stq", [128, 8, SQ], bf16)    # q rope sin (gq + sign folded)
    cpack = din("cpack", [128, 48])         # bq|bk|bo|bvh|rsab|eps (packed)

    outT = nc.dram_tensor("outT", [DIM, SQ], f32, kind="ExternalOutput").ap()

    import os as _os
    _sim = _os.environ.get("KSIM", "0") == "1"
    _dbg = _os.environ.get("KDBG", "0") == "1"
    if _dbg:
        qdbg = nc.dram_tensor("qdbg", [128, 8, SQ], bf16, kind="ExternalOutput").ap()
        ykdbg = nc.dram_tensor("ykdbg", [128, 8, 512], bf16, kind="ExternalOutput").ap()
        rsbdbg = nc.dram_tensor("rsbdbg", [128, CHUNK_TILES], f32, kind="ExternalOutput").ap()
        ptdbg = nc.dram_tensor("ptdbg", [128, SQ], bf16, kind="ExternalOutput").ap()
        ddbg = nc.dram_tensor("ddbg", [128, 8, 4], f32, kind="ExternalOutput").ap()
    qcat = nc.dram_tensor("qcat", [DIM, QS], bf16)
    qcat_sh = nc.dram_tensor("qcat_sh", [N_CORES, DIM, QS], bf16,
                             addr_space="Shared")
    cat = nc.dram_tensor("cat", [DIM + HEADS, SQ], f32)
    cat_sh = nc.dram_tensor("cat_sh", [DIM + HEADS, SQ], f32,
                            addr_space="Shared")

    with tile.TileContext(nc) as tc:
        ctx_pools = []

        def pool(name, bufs, space=None):
            kw = dict(name=name, bufs=bufs)
            if space:
                kw["space"] = space
            p = tc.tile_pool(**kw)
            ctx_pools.append(p)
            return p.__enter__()

        consts = pool("consts", 1)
        resid = pool("resid", 1)
        pp = pool("pp", 3, space="PSUM")          # projection transients
        ppa = pool("ppa", 2, space="PSUM")        # scores
        ppn = pool("ppn", 2, space="PSUM")        # attnV accumulation / Q ssq
        ppd = pool("ppd", 1, space="PSUM")        # den + per-chunk ssq

        # ---- all small constants in one DMA ----
        cp = consts.tile([128, 48], f32)
        bq_s = cp[:, 0:8]
        bk_s = cp[:, 8:16]
        bo_s2 = cp[:, 16:24]
        bvh_s2 = cp[:, 24:32]
        rsab_s = cp[:, 32:34]
        epsq_s = cp[0:1, 34:35]
        ones_s = consts.tile([128, 1], bf16)
        nc.vector.memset(ones_s[:], 1.0)
        qT = resid.tile([128, N_CORES, 8, QS], bf16)  # gathered Q [p,core,o,j]
        den_sb = resid.tile([128, HEADS, 4], f32)
        pid = nc.sync.partition_id()
        qoff = pid * QS

        # single bank, single whole-bank clear (chunk 0's first ssq MM):
        # den in cols 0:32 (never uses start=True; accumulates into the
        # pending-zero region), per-chunk ssq in disjoint cols 32+4ci..
        denbank = ppd.tile([128, 128], f32, tag="den")
        den_ps = denbank[:, 0:32]

        # ---- DMA head ordering: wk (per-o) + chunk0 first so the tensor
        # engine starts on K-projection ~7us in; xT/Wq stream during it ----
        kpool = pool("kpool", 3)
        qpool_cm = tc.tile_pool(name="qpool", bufs=1)
        qpool = qpool_cm.__enter__()
        wk_s = resid.tile([128, 8, 8, 128], bf16)   # [p, o, i, m]
        for o in range(2):
            nc.sync.dma_start(wk_s[:, o, :, :], wk[:, o, :, :])

        # chunk layout
        csizes = [2] + [4] * ((TT - 8) // 4) + [2, 2, 1, 1]
        assert sum(csizes) == TT
        nch = len(csizes)
        cstarts = [sum(csizes[:i]) for i in range(nch)]

        chunk_tiles = {}

        def emit_chunk_dma(ci):
            ntt = csizes[ci]
            cw = ntt * 128
            c0 = cstarts[ci] * 128
            memt = kpool.tile([128, 8, CHUNK_TILES * 128], bf16, tag="memt")
            nc.sync.dma_start(
                memt[:, :, 0:cw],
                memT[:, c0:c0 + cw].rearrange("(i p) t -> p i t", p=128))
            cs_t = kpool.tile([128, 2, CHUNK_TILES * 128], bf16, tag="cskt")
            nc.sync.dma_start(cs_t[:, :, 0:cw], cskt[:, :, c0:c0 + cw])
            chunk_tiles[ci] = dict(memt=memt, ctk=cs_t[:, 0, :], stk=cs_t[:, 1, :])

        emit_chunk_dma(0)
        nc.sync.dma_start(cp[:], cpack)
        for o in range(2, 8):
            nc.sync.dma_start(wk_s[:, o, :, :], wk[:, o, :, :])
        wv_s = resid.tile([128, 8, DIM], bf16)
        nc.sync.dma_start(wv_s[:], wv)
        xt_s = qpool.tile([128, 8, QS], bf16, tag="xt")
        nc.sync.dma_start(
            xt_s[:],
            xT.rearrange("(i p) q -> p i q", p=128)[:, :, bass.ds(qoff, QS)])
        wq_all = qpool.tile([128, 8, 8, 128], bf16, tag="wq_all")
        nc.sync.dma_start(wq_all[:], wq)
        qpf_cm = tc.tile_pool(name="qpf", bufs=1)
        qpf = qpf_cm.__enter__()
        ctq_all = qpf.tile([128, 8, QS], bf16, tag="ctq_all")
        nc.sync.dma_start(ctq_all[:], ctq[:, :, bass.ds(qoff, QS)])
        stq_all = qpf.tile([128, 8, QS], bf16, tag="stq_all")
        nc.sync.dma_start(stq_all[:], stq[:, :, bass.ds(qoff, QS)])
        mb_s = consts.tile([128, TT], f32)
        nc.sync.dma_start(mb_s[:], mbias)
        if nch > 1:
            emit_chunk_dma(1)
        if nch > 2:
            emit_chunk_dma(2)

        MULT = mybir.AluOpType.mult
        ADD = mybir.AluOpType.add

        def produce_pieces(ci):
            """K proj + rsqrt + rope + V proj for chunk ci, as a list of
            emission callables (interleaved with the previous chunk's
            consume phase)."""
            ntt = csizes[ci]
            cw = ntt * 128
            ct = chunk_tiles[ci]
            memt, ctk_t, stk_t = ct["memt"], ct["ctk"], ct["stk"]
            yk = kpool.tile([128, 8, CHUNK_TILES * 128], bf16, tag="yk")
            ct["kr"] = yk                       # rope overwrites in place
            v_sb = kpool.tile([128, CHUNK_TILES, DIM], bf16, tag="v")
            ct["v"] = v_sb
            rsb_t = kpool.tile([128, CHUNK_TILES], f32, tag="rsbt")
            ct["rsb"] = rsb_t
            pieces = []

            def kproj(o):
                ps_y = pp.tile([128, 512], f32, tag="ps")
                for i in range(8):
                    nc.tensor.matmul(ps_y[:, 0:cw], wk_s[:, o, i, :],
                                     memt[:, i, 0:cw],
                                     start=(i == 0), stop=(i == 7))
                nc.vector.tensor_scalar_add(yk[:, o, 0:cw], ps_y[:, 0:cw],
                                            bk_s[:, o:o + 1])
                ysq = kpool.tile([128, CHUNK_TILES * 128], bf16, tag="ysq")
                nc.gpsimd.tensor_mul(ysq[:, 0:cw], yk[:, o, 0:cw],
                                     yk[:, o, 0:cw])
                sb = 32 + 4 * ci
                for tt in range(ntt):
                    # start=True only on the program's very first ssq MM
                    # (clears the whole bank once); everything else in this
                    # bank accumulates into the pending-zero region.
                    nc.tensor.matmul(
                        denbank[:, sb + tt:sb + tt + 1],
                        ysq[:, tt * 128:(tt + 1) * 128], ones_s[:],
                        start=(ci == 0 and o == 0 and tt == 0), stop=(o == 7))

            def rsqrt():
                # v = ssq/8 + 128*eps;  y = a - b*v  (host-fitted affine seed)
                # then two Newton steps y <- y*(1.5 - 0.5*v*y^2) on DVE.
                vt = kpool.tile([128, CHUNK_TILES], f32, tag="vt")
                sb = 32 + 4 * ci
                nc.vector.tensor_scalar(vt[:, 0:ntt], denbank[:, sb:sb + ntt],
                                        0.125, 128.0 * EPS, MULT, ADD)
                nc.vector.tensor_scalar(rsb_t[:, 0:ntt], vt[:, 0:ntt],
                                        rsab_s[:, 1:2], rsab_s[:, 0:1],
                                        MULT, ADD)
                tn = kpool.tile([128, CHUNK_TILES], f32, tag="tn")
                for _ in range(2):
                    nc.vector.tensor_mul(tn[:, 0:ntt], rsb_t[:, 0:ntt],
                                         rsb_t[:, 0:ntt])
                    nc.vector.tensor_mul(tn[:, 0:ntt], tn[:, 0:ntt],
                                         vt[:, 0:ntt])
                    nc.vector.tensor_scalar(tn[:, 0:ntt], tn[:, 0:ntt],
                                            -0.5, 1.5, MULT, ADD)
                    nc.vector.tensor_mul(rsb_t[:, 0:ntt], rsb_t[:, 0:ntt],
                                         tn[:, 0:ntt])
                if _dbg and ci == 0:
                    nc.sync.dma_start(rsbdbg, rsb_t[:])

            def rope(o):
                shf = kpool.tile([128, CHUNK_TILES * 128], bf16, tag="shf")
                nc.vector.stream_shuffle(shf[:, 0:cw], yk[:, o, 0:cw],
                                         SWAP_MASK)
                t1 = kpool.tile([128, CHUNK_TILES * 128], bf16, tag="t1")
                nc.vector.tensor_mul(t1[:, 0:cw], yk[:, o, 0:cw],
                                     ctk_t[:, 0:cw])
                t2 = kpool.tile([128, CHUNK_TILES * 128], bf16, tag="t2")
                nc.vector.tensor_mul(t2[:, 0:cw], shf[:, 0:cw],
                                     stk_t[:, 0:cw])
                nc.vector.tensor_add(yk[:, o, 0:cw], t1[:, 0:cw], t2[:, 0:cw])

            def vproj(tt):
                for oh in range(2):
                    ps_v = pp.tile([128, 512], f32, tag="ps")
                    for i in range(8):
                        nc.tensor.matmul(
                            ps_v[:], memt[:, i, tt * 128:(tt + 1) * 128],
                            wv_s[:, i, oh * 512:(oh + 1) * 512],
                            start=(i == 0), stop=(i == 7))
                    nc.scalar.activation(v_sb[:, tt, oh * 512:(oh + 1) * 512],
                                         ps_v[:], AF.Identity)

            pieces.append(lambda: (kproj(0), kproj(1)))
            pieces.append(lambda: (kproj(2), kproj(3)))
            pieces.append(lambda: (kproj(4), kproj(5)))
            pieces.append(lambda: (kproj(6), kproj(7), rsqrt()))
            pieces.append(lambda: (rope(0), rope(1), rope(2), rope(3)))
            pieces.append(lambda: (rope(4), rope(5), rope(6), rope(7)))
            pieces.append(lambda: tuple(vproj(t) for t in range(0, min(2, ntt))))
            if ntt > 2:
                pieces.append(lambda: tuple(vproj(t) for t in range(2, ntt)))
            return pieces

        def consume_pieces(ci):
            """scores -> exp -> attnV + den for chunk ci."""
            ntt = csizes[ci]
            ct0 = cstarts[ci]
            ct = chunk_tiles[ci]
            kr, v_sb, rsb_t = ct["kr"], ct["v"], ct["rsb"]
            state = {}
            last = ci == nch - 1

            def emit_numden(h, pts):
                ps_n = ppn.tile([128, SQ], f32, tag="psn")
                for tt in range(ntt):
                    nc.tensor.matmul(
                        ps_n[:], v_sb[:, tt, h * 128:(h + 1) * 128],
                        pts[tt][:], start=(tt == 0), stop=(tt == ntt - 1))
                for tt in range(ntt):
                    for qs in range(4):
                        nc.tensor.matmul(
                            den_ps[:, h * 4 + qs:h * 4 + qs + 1],
                            pts[tt][:, qs * 128:(qs + 1) * 128], ones_s[:],
                            start=False,
                            stop=(ci == nch - 1 and h == 7 and tt == ntt - 1
                                  and qs == 3))
                if ci == 0:
                    nc.vector.tensor_copy(nacc[:, h, :], ps_n[:])
                else:
                    nc.vector.tensor_add(nacc[:, h, :], nacc[:, h, :], ps_n[:])
                if last:
                    # numerator for this head is final: ship its cat slice now
                    if _sim:
                        nc.sync.dma_start(cat_sh[h * 128:(h + 1) * 128, :],
                                          nacc[:, h, :])
                    else:
                        nc.sync.dma_start(cat[h * 128:(h + 1) * 128, :],
                                          nacc[:, h, :])

            def head(h):
                pts = []
                for tt in range(ntt):
                    gtt = ct0 + tt
                    ps_s = ppa.tile([128, SQ], f32, tag="psa")
                    nc.tensor.matmul(ps_s[:], kr[:, h, tt * 128:(tt + 1) * 128],
                                     qT[:, :, h, :])
                    pt = ppool.tile([128, SQ], bf16, tag="pt")
                    nc.scalar.activation(pt[:], ps_s[:], AF.Exp,
                                         bias=mb_s[:, gtt:gtt + 1],
                                         scale=rsb_t[:, tt:tt + 1])
                    if _dbg and ci == 0 and h == 0 and tt == 0:
                        nc.sync.dma_start(ptdbg, pt[:])
                    pts.append(pt)
                prev = state.pop("prev", None)
                if prev is not None:
                    emit_numden(*prev)
                state["prev"] = (h, pts)

            def drain():
                prev = state.pop("prev", None)
                if prev is not None:
                    emit_numden(*prev)

            def attn_piece():
                prev = state.pop("prev", None)
                if prev is not None:
                    emit_numden(*prev)

            def head_scores(h):
                pts = []
                for tt in range(ntt):
                    gtt = ct0 + tt
                    ps_s = ppa.tile([128, SQ], f32, tag="psa")
                    nc.tensor.matmul(ps_s[:], kr[:, h, tt * 128:(tt + 1) * 128],
                                     qT[:, :, h, :])
                    pt = ppool.tile([128, SQ], bf16, tag="pt")
                    nc.scalar.activation(pt[:], ps_s[:], AF.Exp,
                                         bias=mb_s[:, gtt:gtt + 1],
                                         scale=rsb_t[:, tt:tt + 1])
                    pts.append(pt)
                state["prev"] = (h, pts)

            pieces = []
            for h in range(8):
                pieces.append(lambda h=h: head_scores(h))
                pieces.append(attn_piece)
            pieces.append(drain)
            return pieces

        for piece in produce_pieces(0):
            piece()

        # =========== Q phase ===========
        yq = qpool.tile([128, 8, QS], bf16, tag="yq")
        qT_own = qpool.tile([128, 8, QS], bf16, tag="qown")
        psq = ppn.tile([128, SQ], f32, tag="psn")   # partition-0 row holds ssq
        for o in range(8):
            ps_q = pp.tile([128, SQ], f32, tag="ps")
            for i in range(8):
                nc.tensor.matmul(ps_q[:, 0:QS], wq_all[:, i, o, :],
                                 xt_s[:, i, :], start=(i == 0), stop=(i == 7))
            nc.scalar.activation(yq[:, o, :], ps_q[:, 0:QS], AF.Identity,
                                 bias=bq_s[:, o:o + 1])
            ysq = qpool.tile([128, QS], bf16, tag="ysq")
            nc.vector.tensor_mul(ysq[:], yq[:, o, :], yq[:, o, :])
            nc.tensor.matmul(psq[0:1, 0:QS], ones_s[:], ysq[:],
                             start=(o == 0), stop=(o == 7))
        sq_q = qpool.tile([1, QS], f32, tag="sqr")
        nc.scalar.activation(sq_q[:], psq[0:1, 0:QS], AF.Sqrt,
                             bias=epsq_s[:], scale=1.0 / DIM)
        rs_q = qpool.tile([1, QS], f32, tag="rs")
        nc.vector.reciprocal(rs_q[:], sq_q[:])
        rsb_q = qpool.tile([128, QS], f32, tag="rsb")
        nc.gpsimd.partition_broadcast(rsb_q[:], rs_q[:])
        for o in range(8):
            shf = qpool.tile([128, QS], bf16, tag="shf")
            nc.vector.stream_shuffle(shf[:], yq[:, o, :], SWAP_MASK)
            t1 = qpool.tile([128, QS], bf16, tag="t1")
            nc.vector.tensor_mul(t1[:], yq[:, o, :], ctq_all[:, o, :])
            t2 = qpool.tile([128, QS], bf16, tag="t2")
            nc.vector.tensor_mul(t2[:], shf[:], stq_all[:, o, :])
            nc.vector.tensor_add(t1[:], t1[:], t2[:])
            nc.vector.tensor_mul(qT_own[:, o, :], t1[:], rsb_q[:])
        # gather all cores' q blocks (block c == original queries c*QS..)
        if _sim:
            nc.sync.dma_start(qcat_sh[bass.ds(pid, 1), :, :].rearrange(
                "c (o p) j -> p c o j", p=128), qT_own[:])
        else:
            nc.sync.dma_start(qcat.rearrange("(o p) q -> p o q", p=128),
                              qT_own[:])
            nc.gpsimd.collective_compute(
                "AllGather", mybir.AluOpType.bypass,
                replica_groups=[list(range(N_CORES))],
                ins=[qcat[:]], outs=[qcat_sh[:]])
        nc.sync.dma_start(
            qT[:], qcat_sh.rearrange("c (o p) j -> p c o j", p=128))
        qpf_cm.__exit__(None, None, None)
        qpool_cm.__exit__(None, None, None)
        ppool = pool("ppool", 24)
        late = pool("late", 1)
        nacc = late.tile([128, 8, SQ], f32)     # numerator accumulator

        prevC = consume_pieces(0)
        for ci in range(1, nch):
            if ci + 2 < nch:
                emit_chunk_dma(ci + 2)
            if ci == 1:
                # out-projection weights: load mid-loop so the tail needn't wait
                wo_s = late.tile([128, 8, 8, 128], bf16)
                nc.sync.dma_start(wo_s[:], wo)
            P = produce_pieces(ci)
            C = prevC
            # proportional interleave (consume leads)
            nC = max(len(C), 1)
            acc = 0.0
            pi = 0
            for k, c in enumerate(C):
                # ramp: produce denser later in the round (Act backlog grows)
                acc += len(P) * 2.0 * (k + 0.5) / (nC * nC)
                while pi < len(P) and acc >= pi + 1:
                    P[pi]()
                    pi += 1
                c()
            while pi < len(P):
                P[pi]()
                pi += 1
            prevC = consume_pieces(ci)
        for c in prevC:
            c()
        if nch == 1:
            wo_s = late.tile([128, 8, 8, 128], bf16)
            nc.sync.dma_start(wo_s[:], wo)

        # =========== reduce across cores ===========
        nc.vector.tensor_copy(den_sb[:], den_ps[:])
        if _dbg:
            nc.sync.dma_start(ddbg, den_sb[:])
        if _sim:
            nc.sync.dma_start(
                cat_sh[DIM:DIM + HEADS, :].rearrange("h (qs p) -> p h qs", p=128),
                den_sb[:])
        else:
            nc.sync.dma_start(
                cat[DIM:DIM + HEADS, :].rearrange("h (qs p) -> p h qs", p=128),
                den_sb[:])
            nc.gpsimd.collective_compute(
                "AllReduce", mybir.AluOpType.add,
                replica_groups=[list(range(N_CORES))],
                ins=[cat[:]], outs=[cat_sh[:]])

        # =========== per-core output projection on its query slice ===========
        tail = pool("tail", 1)
        nred = tail.tile([128, 8, QS], f32)
        dred = tail.tile([1, HEADS, QS], f32)
        nc.sync.dma_start(
            nred[:],
            cat_sh[0:DIM, bass.ds(qoff, QS)].rearrange("(h p) q -> p h q", p=128))
        nc.sync.dma_start(dred[:], cat_sh[DIM:DIM + HEADS, bass.ds(qoff, QS)])
        rd = tail.tile([1, HEADS, QS], f32)
        nc.vector.reciprocal(rd[:], dred[:])
        nsc = tail.tile([128, 8, QS], bf16)
        rdb = tail.tile([128, 8, QS], f32, tag="rdb")
        nc.gpsimd.partition_broadcast(rdb[:], rd[0:1, :, :])
        nc.vector.tensor_mul(nsc[:], nred[:], rdb[:])
        for h in range(8):
            nc.vector.tensor_scalar_add(nsc[:, h, :], nsc[:, h, :],
                                        bvh_s2[:, h:h + 1])
        out_sb = tail.tile([128, 8, QS], f32)
        for e in range(8):
            ps_o = pp.tile([128, QS], f32, tag="ps")
            for o in range(8):
                nc.tensor.matmul(ps_o[:], wo_s[:, o, e, :], nsc[:, o, :],
                                 start=(o == 0), stop=(o == 7))
            nc.scalar.activation(out_sb[:, e, :], ps_o[:], AF.Identity,
                                 bias=bo_s2[:, e:e + 1])
        nc.sync.dma_start(
            outT.rearrange("(e p) q -> p e q", p=128)[:, :, 0:QS], out_sb[:])

        for p in reversed(ctx_pools):
            p.__exit__(None, None, None)

    nc.compile()
    _cache[key] = nc
    return nc


def _prep(TT, x, mem, mask, cos_q, sin_q, cos_k, sin_k,
          Wq, bq, Wk, bk, Wv, bv, Wo, bo, gq, gk):
    import ml_dtypes
    f = np.float32
    bf = ml_dtypes.bfloat16
    SKC = TT * 128
    CAP = N_CORES * SKC
    x = np.asarray(x, f).reshape(SQ, DIM)
    mem = np.asarray(mem, f).reshape(SK, DIM)
    mask = np.asarray(mask).reshape(SK)
    cos_q = np.asarray(cos_q, f)
    sin_q = np.asarray(sin_q, f)
    cos_k = np.asarray(cos_k, f)
    sin_k = np.asarray(sin_k, f)
    Wq, Wk, Wv, Wo = (np.asarray(w, f) for w in (Wq, Wk, Wv, Wo))
    bq, bk, bv, bo, gq, gk = (np.asarray(v, f) for v in (bq, bk, bv, bo, gq, gk))

    if not np.allclose(gk, 1.0):
        gkp = gk.reshape(-1, 2)
        assert np.allclose(gkp[:, 0], gkp[:, 1]), "unsupported non-pairwise gk"

    # mask compaction: keep only unmasked keys
    sel = np.flatnonzero(mask)
    n = sel.size
    assert n <= CAP, f"unmasked keys {n} exceed capacity {CAP}"
    memc = mem[sel]
    cos_kc = cos_k[sel]
    sin_kc = sin_k[sel]

    def tile_w(WT):  # [1024,1024] (in,out of W.T) -> [p, i, o, m]
        return np.ascontiguousarray(
            WT.reshape(8, 128, 8, 128).transpose(1, 0, 2, 3)).astype(bf)

    ii = np.arange(128)
    jj = ii // 2
    partner = ii ^ 1
    # sign of the swapped term: even in-head dims get -sin, odd get +sin
    sgn = np.where(ii % 2 == 0, -1.0, 1.0).astype(f)

    # fold gq (and pairwise gk) into the q rope tables; sin pairs with
    # partner's gq; swap sign folded into the sin tables
    gq_t = (gq * gk).reshape(8, 128)
    gq_sin = (gq.reshape(8, 128)[:, partner] * gk.reshape(8, 128))
    cq = cos_q[:, jj].T                # [128, SQ]
    sq = sin_q[:, jj].T * sgn[:, None]
    ctq_h = np.ascontiguousarray(
        (cq[None, :, :] * gq_t[:, :, None]).transpose(1, 0, 2)).astype(bf)
    stq_h = np.ascontiguousarray(
        (sq[None, :, :] * gq_sin[:, :, None]).transpose(1, 0, 2)).astype(bf)

    # rsqrt Newton affine seed from the expected per-key ssq scale:
    # v = ssq/8 + 128*eps with E[ssq] ~ |Wk|_F^2 var(mem) + |bk|^2
    e_ssq = float((Wk ** 2).sum()) * float(mem.var()) + float((bk ** 2).sum())
    v0 = max(e_ssq * 0.125 + 128.0 * EPS, 1e-6)
    y0 = 1.0 / np.sqrt(v0)
    cpack_h = np.zeros((128, 48), f)
    cpack_h[:, 0:8] = bq.reshape(8, 128).T
    cpack_h[:, 8:16] = bk.reshape(8, 128).T
    cpack_h[:, 16:24] = bo.reshape(8, 128).T
    cpack_h[:, 24:32] = bv.reshape(8, 128).T
    cpack_h[:, 32] = 1.5 * y0        # a:  y1 = a + b*v with b negative
    cpack_h[:, 33] = -0.5 * y0 ** 3
    cpack_h[:, 34] = EPS

    shared = {
        "xT": np.ascontiguousarray(x.T).astype(bf),
        "wq": tile_w(Wq.T),
        "wk": np.ascontiguousarray(tile_w(Wk.T).transpose(0, 2, 1, 3)),
        "wo": tile_w(Wo.T),
        "wv": np.ascontiguousarray(
            Wv.T.reshape(8, 128, DIM).transpose(1, 0, 2)).astype(bf),
        "ctq": ctq_h, "stq": stq_h,
        "cpack": cpack_h,
    }

    memT_full = np.zeros((DIM, CAP), bf)
    memT_full[:, :n] = memc.T.astype(bf)
    cskt_full = np.zeros((HD, 2, CAP), bf)
    cskt_full[:, 0, :n] = cos_kc[:, jj].T
    cskt_full[:, 1, :n] = sin_kc[:, jj].T * sgn[:, None]
    mb_full = np.full(CAP, NEG, f)
    mb_full[:n] = BSHIFT

    in_maps = []
    for c in range(N_CORES):
        s = slice(c * SKC, (c + 1) * SKC)
        m = dict(shared)
        m["memT"] = np.ascontiguousarray(memT_full[:, s])
        m["cskt"] = np.ascontiguousarray(cskt_full[:, :, s])
        m["mbias"] = np.ascontiguousarray(mb_full[s].reshape(TT, 128).T)
        in_maps.append(m)
    return in_maps


def _tt_for(mask):
    n = int(np.count_nonzero(np.asarray(mask).reshape(-1)))
    for tt in (16, 24, 32):
        if n <= N_CORES * tt * 128:
            return tt
    raise AssertionError("mask popcount exceeds max capacity")


def kernel(**inputs):
    from concourse.bass_utils import run_bass_kernel_spmd
    TT = _tt_for(inputs["mask"])
    nc = _build(TT)
    in_maps = _prep(TT, **inputs)
    res = run_bass_kernel_spmd(nc, in_maps, list(range(N_CORES)))
    parts = [res.results[c]["outT"][:, 0:QS].T for c in range(N_CORES)]
    out = np.concatenate(parts, axis=0)
    return out[None].astype(np.float32)
